# revision 30
# baseline (speedup 1.0000x reference)
"""GATv2Conv-with-edge-features Trainium2 kernel (8-core SPMD, edge-sharded by dst).

Self-contained: hardcodes problem shapes (N=50000 nodes, E=800000 edges,
128 feat, 8 heads x 16). Core k owns dst nodes [6250k, 6250(k+1)) and the
edges pointing into them. Edges are sorted by dst and packed into tiles of
<=128 edges covering <=32 consecutive dst nodes; tile windows PARTITION the
local node range so every node (incl. degree-0) has exactly one slot.

Single fused loop over chunks of 8 tiles (1024 edges), software-pipelined
with a 2-chunk skew so every engine streams:
  stage A (chunk c):   one mega-DMA (xs16/ef16/xd8/S16/xdn8 feature-major),
                       T = xs@Ws + xd@Wd + ef@We per tile (fp16 matmuls +
                       one fp8 DoubleRow for the hi/lo-split dst term),
                       lin = (0.6 attn)^T T via tiny matmuls,
                       T16 = Copy(T_ps) on Act, |T| via bitwise-and (DVE 4x),
                       F = |T| * (0.4 attn) (DVE 2x, d-major columns),
  stage B (chunk c-2): score = lin + sum_d F via 16 tiny identity-matmul
                       folds on PE (no vector tree), ex = Exp(score-4) into
                       the z-columns of msg, msg = T16*ex (DVE 4 tiles /
                       Pool 4 tiles), scatter S^T@msg into U|z PSUM (8
                       exclusive regions), fdst per slot from xdn via the
                       same fp8 DR matmul (exact cancellation), then
                       out = relu((U - fdst*z) / max(z,2^-14)) in fp16 and
                       one 512B-descriptor DMA of slot-ordered rows.
Host does layout only: pack/sort/gather into the mega buffer, and scatter
the slot-ordered output rows back to node order (numpy indexing).
"""
import numpy as np
import ml_dtypes

import concourse.bacc as bacc
import concourse.tile as tile
import concourse.mybir as mybir
from concourse.bass_utils import run_bass_kernel_spmd

N_NODES = 50000
N_CORES = 8
N_LOCAL = N_NODES // N_CORES          # 6250
IN_FEAT = 128
HEADS = 8
HEAD_DIM = 16
TILE_E = 128
TILE_W = 32
CH_TILES = 8                          # tiles per chunk
EXP_SHIFT = 4.0
EPS_Z = 2.0 ** -14                    # fp16-safe softmax-denominator floor
P = 128
FP = mybir.dt.float32
F16 = mybir.dt.float16
F8 = mybir.dt.float8e4
U8 = mybir.dt.uint8
NP8 = ml_dtypes.float8_e4m3
NP16 = np.float16

TILE_BYTES = 704                      # xs16 256 | ef16 256 | xd8 128 | S16 64
XDN_BYTES = 3 * P                     # 384: 3 col-groups x (3x32 slots + pad)
CH_BYTES = CH_TILES * TILE_BYTES + XDN_BYTES   # 5888
SKEW = 2

# d-major output-feature permutation: T column j = feature PJ[j]
PJ = np.array([h * HEAD_DIM + d for d in range(HEAD_DIM) for h in range(HEADS)])


# ---------------------------------------------------------------- host prep

def _pack_core(dst_local, n_local):
    """Best-fit-decreasing bins: <=TILE_E edges and <=TILE_W nodes per tile.
    Windows need not be consecutive; the host owns the slot->node map."""
    import bisect
    deg = np.bincount(dst_local, minlength=n_local)
    order = np.argsort(-deg, kind="stable")
    bins_nodes = []          # list of node-lists
    bins_edges = []          # remaining edge capacity per bin
    open_caps = []           # sorted (rem_edges, bin_idx) for bins w/ node room
    for n in order:
        d = int(deg[n])
        assert d <= TILE_E
        pos = bisect.bisect_left(open_caps, (d, -1))
        if pos < len(open_caps):
            rem, b = open_caps.pop(pos)
            bins_nodes[b].append(n)
            bins_edges[b] = rem - d
            if len(bins_nodes[b]) < TILE_W:
                bisect.insort(open_caps, (rem - d, b))
        else:
            b = len(bins_nodes)
            bins_nodes.append([n])
            bins_edges.append(TILE_E - d)
            if TILE_W > 1:
                bisect.insort(open_caps, (TILE_E - d, b))
    t_of_node = np.zeros(n_local, np.int64)
    s_of_node = np.zeros(n_local, np.int64)
    for t, nodes in enumerate(bins_nodes):
        idx = np.asarray(nodes)
        t_of_node[idx] = t
        s_of_node[idx] = np.arange(len(idx))
    tile_cnt = np.array([int(deg[np.asarray(nodes)].sum())
                         for nodes in bins_nodes])
    assert (tile_cnt <= TILE_E).all()
    return bins_nodes, tile_cnt, t_of_node, s_of_node


def _prep_cores(x, efeat, src, dst, W_src, b_src, W_dst, b_dst, W_edge, attn):
    x = np.ascontiguousarray(np.asarray(x, np.float32))
    efeat = np.asarray(efeat, np.float32)
    src = np.asarray(src).astype(np.int64)
    dst = np.asarray(dst).astype(np.int64)
    W_src = np.asarray(W_src, np.float32)
    W_dst = np.asarray(W_dst, np.float32)
    W_edge = np.asarray(W_edge, np.float32)
    attn = np.asarray(attn, np.float32)
    assert np.abs(np.asarray(b_src)).max() == 0
    assert np.abs(np.asarray(b_dst)).max() == 0

    x16 = x.astype(NP16)
    x8 = x.astype(NP8)
    ef16 = efeat.astype(NP16)

    per_core = []
    core_T = []
    for k in range(N_CORES):
        lo = k * N_LOCAL
        eidx = np.nonzero((dst >= lo) & (dst < lo + N_LOCAL))[0]
        dl = (dst[eidx] - lo).astype(np.int64)
        order = np.argsort(dl, kind="stable")
        eidx, dl = eidx[order], dl[order]
        per_core.append((eidx, dl) + _pack_core(dl, N_LOCAL))
        core_T.append(len(per_core[-1][2]) - 1)

    T_tiles = max(core_T)
    T_tiles = ((T_tiles + CH_TILES - 1) // CH_TILES) * CH_TILES
    n_ch = T_tiles // CH_TILES

    # weights: output columns permuted to d-major
    WsT16 = np.ascontiguousarray(W_src[PJ].T.astype(NP16))      # [128,128]
    WeT16 = np.ascontiguousarray(W_edge[PJ].T.astype(NP16))
    WdT = W_dst[PJ].T                                           # fp32
    Wd_hi = WdT.astype(NP8)
    Wd_lo = (WdT - Wd_hi.astype(np.float32)).astype(NP8)
    Wd8p = np.ascontiguousarray(np.concatenate([Wd_hi, Wd_lo], axis=1))

    attn_flat = np.zeros((IN_FEAT, HEADS), np.float32)
    for h in range(HEADS):
        attn_flat[h * HEAD_DIM:(h + 1) * HEAD_DIM, h] = attn[h]
    wts16 = np.ascontiguousarray((W_src.T @ (0.6 * attn_flat)).astype(NP16))
    wte16 = np.ascontiguousarray((W_edge.T @ (0.6 * attn_flat)).astype(NP16))
    wtd32 = W_dst.T @ (0.6 * attn_flat)
    wtd_hi = wtd32.astype(NP8)
    wtd_lo = (wtd32 - wtd_hi.astype(np.float32)).astype(NP8)
    wtd8p = np.ascontiguousarray(np.concatenate([wtd_hi, wtd_lo], axis=1))

    arep16 = np.ascontiguousarray(np.broadcast_to(
        (0.4 * attn.T).reshape(1, IN_FEAT), (P, IN_FEAT)).astype(NP16))
    ident16 = np.eye(P, dtype=NP16)

    in_maps = []
    node_maps = []
    for k in range(N_CORES):
        eidx, dl, bins_nodes, tcnt, t_of_node, s_of_node = per_core[k]

        # per-node edge ranges in the dst-sorted edge order
        deg = np.bincount(dl, minlength=N_LOCAL)
        starts = np.concatenate([[0], np.cumsum(deg)[:-1]])

        mega = np.zeros((P, n_ch * CH_BYTES), np.uint8)
        for t, nodes in enumerate(bins_nodes):
            c, tl = t // CH_TILES, t % CH_TILES
            base = c * CH_BYTES + tl * TILE_BYTES
            e_ids = np.concatenate(
                [eidx[starts[n]:starts[n] + deg[n]] for n in nodes]) \
                if nodes else np.zeros(0, np.int64)
            slots = np.concatenate(
                [np.full(deg[n], si) for si, n in enumerate(nodes)]) \
                if nodes else np.zeros(0, np.int64)
            d_loc = np.concatenate(
                [np.full(deg[n], n) for n in nodes]) \
                if nodes else np.zeros(0, np.int64)
            cnt = len(e_ids)
            if cnt:
                mega[:, base:base + 256].view(NP16)[:, :cnt] = \
                    x16[src[e_ids]].T
                mega[:, base + 256:base + 512].view(NP16)[:, :cnt] = \
                    ef16[e_ids].T
                mega[:, base + 512:base + 640].view(NP8)[:, :cnt] = \
                    x8[d_loc + k * N_LOCAL].T
                sview = mega[:, base + 640:base + 704].view(NP16)
                sview[np.arange(cnt), slots] = NP16(1.0)
            # per-slot node features for the fdst recompute
            xb = c * CH_BYTES + CH_TILES * TILE_BYTES \
                + (tl // 3) * P + (tl % 3) * TILE_W
            if nodes:
                mega[:, xb:xb + len(nodes)].view(NP8)[:, :len(nodes)] = \
                    x8[k * N_LOCAL + np.asarray(nodes)].T

        in_maps.append(dict(
            mega_in=mega, WsT16=WsT16, WeT16=WeT16, Wd8p=Wd8p,
            wts16=wts16, wte16=wte16, wtd8p=wtd8p,
            arep16=arep16, ident16=ident16,
        ))
        node_maps.append((t_of_node, s_of_node))
    return in_maps, node_maps, T_tiles


# ------------------------------------------------------------- bass program

def build_program(T_tiles):
    nc = bacc.Bacc("TRN2", target_bir_lowering=False, debug=False,
                   num_devices=N_CORES)
    n_ch = T_tiles // CH_TILES

    mega_d = nc.dram_tensor("mega_in", [P, n_ch * CH_BYTES], U8,
                            kind="ExternalInput")
    WsT_d = nc.dram_tensor("WsT16", [P, IN_FEAT], F16, kind="ExternalInput")
    WeT_d = nc.dram_tensor("WeT16", [P, IN_FEAT], F16, kind="ExternalInput")
    Wd8_d = nc.dram_tensor("Wd8p", [P, 2 * IN_FEAT], F8, kind="ExternalInput")
    wts_d = nc.dram_tensor("wts16", [P, HEADS], F16, kind="ExternalInput")
    wte_d = nc.dram_tensor("wte16", [P, HEADS], F16, kind="ExternalInput")
    wtd_d = nc.dram_tensor("wtd8p", [P, 2 * HEADS], F8, kind="ExternalInput")
    arep_d = nc.dram_tensor("arep16", [P, IN_FEAT], F16, kind="ExternalInput")
    ident_d = nc.dram_tensor("ident16", [P, P], F16, kind="ExternalInput")
    out_d = nc.dram_tensor("out_sl", [96, n_ch * 408], F16,
                           kind="ExternalOutput")

    with tile.TileContext(nc) as tc:
        with tc.tile_pool(name="const", bufs=1) as cb:
            def cload(name, shape, dt, dram):
                t = cb.tile(shape, dt, name=name)
                nc.sync.dma_start(out=t[:], in_=dram[:])
                return t

            WsT = cload("WsT_s", [P, IN_FEAT], F16, WsT_d)
            WeT = cload("WeT_s", [P, IN_FEAT], F16, WeT_d)
            Wd8 = cload("Wd8_s", [P, 2 * IN_FEAT], F8, Wd8_d)
            wts = cload("wts_s", [P, HEADS], F16, wts_d)
            wte = cload("wte_s", [P, HEADS], F16, wte_d)
            wtd = cload("wtd_s", [P, 2 * HEADS], F8, wtd_d)
            arep = cload("arep_s", [P, IN_FEAT], F16, arep_d)
            ident = cload("ident_s", [P, P], F16, ident_d)

            bias4 = cb.tile([P, 1], FP, name="bias4")
            nc.vector.memset(bias4[:], -EXP_SHIFT)

            Wd8s = Wd8[:].rearrange("p (two f) -> p two f", two=2)
            wtds = wtd[:].rearrange("p (two h) -> p two h", two=2)

            with (
                tc.tile_pool(name="meg", bufs=9) as megp,
                tc.tile_pool(name="t16", bufs=4) as t16p,
                tc.tile_pool(name="ff", bufs=4) as ffp,
                tc.tile_pool(name="msg", bufs=4) as msgp,
                tc.tile_pool(name="u16", bufs=4) as u16p,
                tc.tile_pool(name="fin", bufs=4) as finp,
                tc.tile_pool(name="ps_t", bufs=2, space="PSUM") as pst,
                tc.tile_pool(name="ps_ul", bufs=3, space="PSUM") as psul,
                tc.tile_pool(name="ps_fd", bufs=1, space="PSUM") as psfd,
            ):
                megs, t16s, ffs, uls = {}, {}, {}, {}
                msgs, u16s, o16s = {}, {}, {}

                for c in range(n_ch + 5):
                    j = c - 2             # B1: folds/exp/msg/scat/evacU
                    k2 = c - 3            # B2: fdst + tail
                    k3 = c - 4            # out DMA

                    # ---- deferred slot-row writeback (data ready last iter)
                    if 0 <= k3 < n_ch:
                        nc.scalar.dma_start(
                            out=out_d[:, k3 * 408:(k3 + 1) * 408],
                            in_=o16s.pop(k3)[0:96, :])

                    # ---- mega prefetch (2 iterations ahead)
                    for cc in ([0, 1, 2] if c == 0 else
                               [c + 2] if c + 2 < n_ch else []):
                        meg = megp.tile([P, CH_BYTES], U8, tag="meg")
                        megs[cc] = meg
                        HB = 4 * TILE_BYTES
                        nc.sync.dma_start(
                            out=meg[:, 0:HB],
                            in_=mega_d[:, cc * CH_BYTES:cc * CH_BYTES + HB])
                        nc.sync.dma_start(
                            out=meg[:, HB:CH_BYTES],
                            in_=mega_d[:, cc * CH_BYTES + HB:
                                       (cc + 1) * CH_BYTES])

                    # ---- B1-front: msg mults (chunk j); exp ran last iter
                    if 0 <= j < n_ch:
                        T16j, ULj = t16s.pop(j), uls[j]
                        msg = msgs[j]
                        mv = msg[:].rearrange("p (t f) -> p t f", t=8)
                        exb = mv[:, :, 128:136].unsqueeze(2).to_broadcast(
                            [P, CH_TILES, HEAD_DIM, HEADS])
                        mfeat = msg[:].rearrange(
                            "p (t f) -> p t f", t=8)[:, :, 0:128].rearrange(
                            "p t (d h) -> p t d h", d=16)
                        t16v = T16j[:].rearrange("p (t d h) -> p t d h",
                                                 t=8, d=16)
                        nc.vector.tensor_tensor(
                            out=mfeat[:, 0:3], in0=t16v[:, 0:3],
                            in1=exb[:, 0:3], op=mybir.AluOpType.mult)
                        nc.gpsimd.tensor_tensor(
                            out=mfeat[:, 3:8], in0=t16v[:, 3:8],
                            in1=exb[:, 3:8], op=mybir.AluOpType.mult)

                    # ---- A-compute: T + lin matmuls (chunk c)
                    if c < n_ch:
                        meg = megs[c]
                        T_ps = pst.tile([P, CH_TILES * TILE_E], FP, tag="T")
                        UL = psul.tile([P, 3 * 136 + 64], FP, tag="UL")
                        uls[c] = UL
                        # dummies absorb the psum-free waits so the real
                        # matmuls only wait on the mega DMA
                        nc.tensor.matmul(out=T_ps[:1, 0:1],
                                         lhsT=ident[:, :1], rhs=ident[:, :1],
                                         start=True, stop=True)
                        nc.tensor.matmul(out=UL[:1, 408:409],
                                         lhsT=ident[:, :1], rhs=ident[:, :1],
                                         start=True, stop=True,
                                         skip_group_check=True)
                        for tl in range(CH_TILES):
                            o = tl * TILE_BYTES
                            xs = meg[:, o:o + 256].bitcast(F16)
                            ef = meg[:, o + 256:o + 512].bitcast(F16)
                            xd2 = meg[:, o + 512:o + 640].bitcast(F8) \
                                .unsqueeze(1).to_broadcast([P, 2, TILE_E])
                            ts = slice(tl * TILE_E, (tl + 1) * TILE_E)
                            nc.tensor.matmul(out=T_ps[:, ts], lhsT=xs,
                                             rhs=WsT[:], start=True,
                                             stop=False)
                            nc.tensor.matmul(out=T_ps[:, ts], lhsT=xd2,
                                             rhs=Wd8s, start=False,
                                             stop=False,
                                             perf_mode=mybir.MatmulPerfMode
                                             .DoubleRow)
                            nc.tensor.matmul(out=T_ps[:, ts], lhsT=ef,
                                             rhs=WeT[:], start=False,
                                             stop=True)
                            ls = slice(408 + tl * HEADS,
                                       408 + (tl + 1) * HEADS)
                            nc.tensor.matmul(out=UL[:, ls], lhsT=xs,
                                             rhs=wts[:], start=(tl == 0),
                                             stop=False,
                                             skip_group_check=True)
                            nc.tensor.matmul(out=UL[:, ls], lhsT=xd2,
                                             rhs=wtds, start=False,
                                             stop=False,
                                             perf_mode=mybir.MatmulPerfMode
                                             .DoubleRow,
                                             skip_group_check=True)
                            nc.tensor.matmul(out=UL[:, ls], lhsT=ef,
                                             rhs=wte[:], start=False,
                                             stop=False,
                                             skip_group_check=True)

                    # ---- B1-mid: scatters + U evac to SBUF (chunk j)
                    if 0 <= j < n_ch:
                        megj = megs[j]
                        for tl in range(CH_TILES):
                            sb = tl * TILE_BYTES + 640
                            S1 = megj[:, sb:sb + 64].bitcast(F16)
                            g, o3 = tl // 3, tl % 3
                            nc.tensor.matmul(
                                out=ULj[32 * o3:32 * o3 + 32,
                                        136 * g:136 * g + 136],
                                lhsT=S1, rhs=msg[:, tl * 136:tl * 136 + 136],
                                start=True, stop=True)
                        U16 = u16p.tile([P, 408], F16, tag="U16")
                        u16s[j] = U16
                        nc.scalar.activation(
                            out=U16[:], in_=ULj[:, 0:408],
                            func=mybir.ActivationFunctionType.Copy)
                        uls.pop(j)

                    # ---- B2: fdst matmuls + tail from SBUF (chunk k2)
                    if 0 <= k2 < n_ch:
                        megk = megs.pop(k2)
                        U16k = u16s.pop(k2)
                        msgs.pop(k2)
                        fd_ps = psfd.tile([P, 3 * IN_FEAT], FP, tag="fd")
                        nc.tensor.matmul(out=fd_ps[:1, 0:1],
                                         lhsT=ident[:, :1], rhs=ident[:, :1],
                                         start=True, stop=True)
                        xdn = megk[:, CH_TILES * TILE_BYTES:
                                   CH_TILES * TILE_BYTES + XDN_BYTES] \
                            .bitcast(F8)
                        for g in range(3):
                            xp = xdn[:, P * g:P * g + P] \
                                .unsqueeze(1).to_broadcast([P, 2, P])
                            nc.tensor.matmul(
                                out=fd_ps[:, 128 * g:128 * g + 128],
                                lhsT=xp, rhs=Wd8s, start=True, stop=True,
                                perf_mode=mybir.MatmulPerfMode.DoubleRow)

                        fin = finp.tile([P, 24 + 384 + 408], F16, tag="fin")
                        rz16 = fin[:, 0:24]
                        uo = fin[:, 24:408].rearrange(
                            "p (g d h) -> p g d h", g=3, d=16)
                        o16 = fin[:, 408:816]
                        u16f = U16k[:].rearrange("p (g f) -> p g f", g=3)
                        nc.vector.tensor_scalar(
                            out=rz16.rearrange("p (g h) -> p g h", g=3),
                            in0=u16f[:, :, 128:136],
                            scalar1=EPS_Z, scalar2=None,
                            op0=mybir.AluOpType.max)
                        with nc.allow_low_precision(
                                reason="1/z fits fp16; z >= 2^-14"):
                            nc.vector.reciprocal(out=rz16, in_=rz16)
                        rzb = rz16.rearrange("p (g h) -> p g h", g=3) \
                            .unsqueeze(2).to_broadcast([P, 3, 16, HEADS])
                        nc.vector.tensor_tensor(
                            out=uo,
                            in0=u16f[:, :, 0:128].rearrange(
                                "p g (d h) -> p g d h", d=16),
                            in1=rzb, op=mybir.AluOpType.mult)
                        o16v = o16.rearrange("p (g f) -> p g f", g=3, f=136)
                        nc.vector.tensor_tensor(
                            out=o16v[:, :, 0:128].rearrange(
                                "p g (d h) -> p g d h", d=16),
                            in0=fin[:, 24:408].rearrange(
                                "p (g d h) -> p g d h", g=3, d=16),
                            in1=fd_ps[:].rearrange("p (g d h) -> p g d h",
                                                   g=3, d=16),
                            op=mybir.AluOpType.subtract),
                        nc.vector.tensor_scalar(
                            out=o16v[:, :, 0:128], in0=o16v[:, :, 0:128],
                            scalar1=0.0, scalar2=None,
                            op0=mybir.AluOpType.max)
                        nc.gpsimd.tensor_scalar(
                            out=o16v[:, :, 128:136],
                            in0=u16f[:, :, 128:136],
                            scalar1=0.0, scalar2=None,
                            op0=mybir.AluOpType.add)
                        o16s[k2] = o16

                    # ---- folds + exp for chunk c-1 (end of iteration)
                    fE = c - 1
                    if 0 <= fE < n_ch:
                        FFe = ffs.pop(fE)
                        ULe = uls[fE]
                        line = ULe[:, 408:472]
                        Fv = FFe[:].rearrange("p (t d h) -> p t d h",
                                              t=8, d=16)
                        for d in range(HEAD_DIM):
                            nc.tensor.matmul(
                                out=line, lhsT=ident[:], rhs=Fv[:, :, d, :],
                                start=False, stop=(d == HEAD_DIM - 1),
                                skip_group_check=True)
                        msge = msgp.tile([P, CH_TILES * 136], F16,
                                         tag="msg")
                        msgs[fE] = msge
                        nc.scalar.activation(
                            out=msge[:].rearrange("p (t f) -> p t f",
                                                  t=8)[:, :, 128:136],
                            in_=line.rearrange("p (t h) -> p t h", t=8),
                            func=mybir.ActivationFunctionType.Exp,
                            bias=bias4[:], scale=1.0)
                    # ---- A-evac: T16, |T|, F (chunk c)
                    if c < n_ch:
                        T16 = t16p.tile([P, CH_TILES * TILE_E], F16,
                                        tag="T16")
                        t16s[c] = T16
                        nc.scalar.activation(
                            out=T16[:], in_=T_ps[:],
                            func=mybir.ActivationFunctionType.Copy)
                        FF = ffp.tile([P, CH_TILES * TILE_E], F16, tag="FF")
                        ffs[c] = FF
                        nc.vector.tensor_scalar(
                            out=FF[:].bitcast(mybir.dt.int16),
                            in0=T16[:].bitcast(mybir.dt.int16),
                            scalar1=0x7FFF, scalar2=None,
                            op0=mybir.AluOpType.bitwise_and)
                        nc.vector.tensor_tensor(
                            out=FF[:].rearrange("p (t f) -> p t f", t=8),
                            in0=FF[:].rearrange("p (t f) -> p t f", t=8),
                            in1=arep[:].unsqueeze(1).to_broadcast(
                                [P, CH_TILES, IN_FEAT]),
                            op=mybir.AluOpType.mult)

    nc.compile()
    return nc


_PROGRAM_CACHE = {}


def kernel(**inputs) -> np.ndarray:
    in_maps, node_maps, T_tiles = _prep_cores(**inputs)
    if T_tiles not in _PROGRAM_CACHE:
        _PROGRAM_CACHE[T_tiles] = build_program(T_tiles)
    nc = _PROGRAM_CACHE[T_tiles]
    res = run_bass_kernel_spmd(nc, in_maps, list(range(N_CORES)))

    n_ch = T_tiles // CH_TILES
    inv = np.empty_like(PJ)
    inv[PJ] = np.arange(IN_FEAT)      # vals col j -> feature PJ[j]
    outs = []
    for k in range(N_CORES):
        sl = np.asarray(res.results[k]["out_sl"])      # [96, n_ch*408] f16
        sl = sl.reshape(96, n_ch, 3, 136)
        t_of_node, s_of_node = node_maps[k]
        c = t_of_node // CH_TILES
        tl = t_of_node % CH_TILES
        part = 32 * (tl % 3) + s_of_node
        g = tl // 3
        rows = sl[part, c, g, :]                        # [6250, 136]
        vals = rows[:, 0:128]
        zv = rows[:, (128 + np.arange(IN_FEAT) % 8)]
        vals = np.where(zv > 0, vals, np.float16(0.0))
        outs.append(vals[:, inv].astype(np.float32))
    return np.concatenate(outs, axis=0)


# revision 36
# speedup vs baseline: 1.0429x; 1.0429x over previous
"""GATv2Conv-with-edge-features Trainium2 kernel (8-core SPMD, edge-sharded by dst).

Self-contained: hardcodes problem shapes (N=50000 nodes, E=800000 edges,
128 feat, 8 heads x 16). Core k owns dst nodes [6250k, 6250(k+1)) and the
edges pointing into them. Edges are sorted by dst and packed into tiles of
<=128 edges covering <=32 consecutive dst nodes; tile windows PARTITION the
local node range so every node (incl. degree-0) has exactly one slot.

Single fused loop over chunks of 8 tiles (1024 edges), software-pipelined
with a 2-chunk skew so every engine streams:
  stage A (chunk c):   one mega-DMA (xs16/ef16/xd8/S16/xdn8 feature-major),
                       T = xs@Ws + xd@Wd + ef@We per tile (fp16 matmuls +
                       one fp8 DoubleRow for the hi/lo-split dst term),
                       lin = (0.6 attn)^T T via tiny matmuls,
                       T16 = Copy(T_ps) on Act, |T| via bitwise-and (DVE 4x),
                       F = |T| * (0.4 attn) (DVE 2x, d-major columns),
  stage B (chunk c-2): score = lin + sum_d F via 16 tiny identity-matmul
                       folds on PE (no vector tree), ex = Exp(score-4) into
                       the z-columns of msg, msg = T16*ex (DVE 4 tiles /
                       Pool 4 tiles), scatter S^T@msg into U|z PSUM (8
                       exclusive regions), fdst per slot from xdn via the
                       same fp8 DR matmul (exact cancellation), then
                       out = relu((U - fdst*z) / max(z,2^-14)) in fp16 and
                       one 512B-descriptor DMA of slot-ordered rows.
Host does layout only: pack/sort/gather into the mega buffer, and scatter
the slot-ordered output rows back to node order (numpy indexing).
"""
import numpy as np
import ml_dtypes

import concourse.bacc as bacc
import concourse.tile as tile
import concourse.mybir as mybir
from concourse.bass_utils import run_bass_kernel_spmd

N_NODES = 50000
N_CORES = 8
N_LOCAL = N_NODES // N_CORES          # 6250
IN_FEAT = 128
HEADS = 8
HEAD_DIM = 16
TILE_E = 128
TILE_W = 32
CH_TILES = 8                          # tiles per chunk
EXP_SHIFT = 4.0
EPS_Z = 2.0 ** -14                    # fp16-safe softmax-denominator floor
P = 128
FP = mybir.dt.float32
F16 = mybir.dt.float16
F8 = mybir.dt.float8e4
U8 = mybir.dt.uint8
NP8 = ml_dtypes.float8_e4m3
NP16 = np.float16

TILE_BYTES = 704                      # xs16 256 | ef16 256 | xd8 128 | S16 64
XDN_BYTES = 3 * P                     # 384: 3 col-groups x (3x32 slots + pad)
CH_BYTES = CH_TILES * TILE_BYTES + XDN_BYTES   # 5888
SKEW = 2

# d-major output-feature permutation: T column j = feature PJ[j]
PJ = np.array([h * HEAD_DIM + d for d in range(HEAD_DIM) for h in range(HEADS)])


# ---------------------------------------------------------------- host prep

def _pack_core(dst_local, n_local):
    """Best-fit-decreasing bins: <=TILE_E edges and <=TILE_W nodes per tile.
    Windows need not be consecutive; the host owns the slot->node map."""
    import bisect
    deg = np.bincount(dst_local, minlength=n_local)
    order = np.argsort(-deg, kind="stable")
    bins_nodes = []          # list of node-lists
    bins_edges = []          # remaining edge capacity per bin
    open_caps = []           # sorted (rem_edges, bin_idx) for bins w/ node room
    for n in order:
        d = int(deg[n])
        assert d <= TILE_E
        pos = bisect.bisect_left(open_caps, (d, -1))
        if pos < len(open_caps):
            rem, b = open_caps.pop(pos)
            bins_nodes[b].append(n)
            bins_edges[b] = rem - d
            if len(bins_nodes[b]) < TILE_W:
                bisect.insort(open_caps, (rem - d, b))
        else:
            b = len(bins_nodes)
            bins_nodes.append([n])
            bins_edges.append(TILE_E - d)
            if TILE_W > 1:
                bisect.insort(open_caps, (TILE_E - d, b))
    t_of_node = np.zeros(n_local, np.int64)
    s_of_node = np.zeros(n_local, np.int64)
    for t, nodes in enumerate(bins_nodes):
        idx = np.asarray(nodes)
        t_of_node[idx] = t
        s_of_node[idx] = np.arange(len(idx))
    tile_cnt = np.array([int(deg[np.asarray(nodes)].sum())
                         for nodes in bins_nodes])
    assert (tile_cnt <= TILE_E).all()
    return bins_nodes, tile_cnt, t_of_node, s_of_node


def _prep_cores(x, efeat, src, dst, W_src, b_src, W_dst, b_dst, W_edge, attn):
    x = np.ascontiguousarray(np.asarray(x, np.float32))
    efeat = np.asarray(efeat, np.float32)
    src = np.asarray(src).astype(np.int64)
    dst = np.asarray(dst).astype(np.int64)
    W_src = np.asarray(W_src, np.float32)
    W_dst = np.asarray(W_dst, np.float32)
    W_edge = np.asarray(W_edge, np.float32)
    attn = np.asarray(attn, np.float32)
    assert np.abs(np.asarray(b_src)).max() == 0
    assert np.abs(np.asarray(b_dst)).max() == 0

    x16 = x.astype(NP16)
    x8 = x.astype(NP8)
    ef16 = efeat.astype(NP16)

    per_core = []
    core_T = []
    for k in range(N_CORES):
        lo = k * N_LOCAL
        eidx = np.nonzero((dst >= lo) & (dst < lo + N_LOCAL))[0]
        dl = (dst[eidx] - lo).astype(np.int64)
        order = np.argsort(dl, kind="stable")
        eidx, dl = eidx[order], dl[order]
        per_core.append((eidx, dl) + _pack_core(dl, N_LOCAL))
        core_T.append(len(per_core[-1][2]) - 1)

    T_tiles = max(core_T)
    T_tiles = ((T_tiles + CH_TILES - 1) // CH_TILES) * CH_TILES
    n_ch = T_tiles // CH_TILES

    # weights: output columns permuted to d-major
    WsT16 = np.ascontiguousarray(W_src[PJ].T.astype(NP16))      # [128,128]
    WeT16 = np.ascontiguousarray(W_edge[PJ].T.astype(NP16))
    WdT = W_dst[PJ].T                                           # fp32
    Wd_hi = WdT.astype(NP8)
    Wd_lo = (WdT - Wd_hi.astype(np.float32)).astype(NP8)
    Wd8p = np.ascontiguousarray(np.concatenate([Wd_hi, Wd_lo], axis=1))

    attn_flat = np.zeros((IN_FEAT, HEADS), np.float32)
    for h in range(HEADS):
        attn_flat[h * HEAD_DIM:(h + 1) * HEAD_DIM, h] = attn[h]
    wts16 = np.ascontiguousarray((W_src.T @ (0.6 * attn_flat)).astype(NP16))
    wte16 = np.ascontiguousarray((W_edge.T @ (0.6 * attn_flat)).astype(NP16))
    wtd32 = W_dst.T @ (0.6 * attn_flat)
    wtd_hi = wtd32.astype(NP8)
    wtd_lo = (wtd32 - wtd_hi.astype(np.float32)).astype(NP8)
    wtd8p = np.ascontiguousarray(np.concatenate([wtd_hi, wtd_lo], axis=1))

    arep16 = np.ascontiguousarray(np.broadcast_to(
        (0.4 * attn.T).reshape(1, IN_FEAT), (P, IN_FEAT)).astype(NP16))
    ident16 = np.eye(P, dtype=NP16)

    in_maps = []
    node_maps = []
    for k in range(N_CORES):
        eidx, dl, bins_nodes, tcnt, t_of_node, s_of_node = per_core[k]

        # per-node edge ranges in the dst-sorted edge order
        deg = np.bincount(dl, minlength=N_LOCAL)
        starts = np.concatenate([[0], np.cumsum(deg)[:-1]])

        mega = np.zeros((P, n_ch * CH_BYTES), np.uint8)
        for t, nodes in enumerate(bins_nodes):
            c, tl = t // CH_TILES, t % CH_TILES
            base = c * CH_BYTES + tl * TILE_BYTES
            e_ids = np.concatenate(
                [eidx[starts[n]:starts[n] + deg[n]] for n in nodes]) \
                if nodes else np.zeros(0, np.int64)
            slots = np.concatenate(
                [np.full(deg[n], si) for si, n in enumerate(nodes)]) \
                if nodes else np.zeros(0, np.int64)
            d_loc = np.concatenate(
                [np.full(deg[n], n) for n in nodes]) \
                if nodes else np.zeros(0, np.int64)
            cnt = len(e_ids)
            if cnt:
                mega[:, base:base + 256].view(NP16)[:, :cnt] = \
                    x16[src[e_ids]].T
                mega[:, base + 256:base + 512].view(NP16)[:, :cnt] = \
                    ef16[e_ids].T
                mega[:, base + 512:base + 640].view(NP8)[:, :cnt] = \
                    x8[d_loc + k * N_LOCAL].T
                sview = mega[:, base + 640:base + 704].view(NP16)
                sview[np.arange(cnt), slots] = NP16(1.0)
            # per-slot node features for the fdst recompute
            xb = c * CH_BYTES + CH_TILES * TILE_BYTES \
                + (tl // 3) * P + (tl % 3) * TILE_W
            if nodes:
                mega[:, xb:xb + len(nodes)].view(NP8)[:, :len(nodes)] = \
                    x8[k * N_LOCAL + np.asarray(nodes)].T

        in_maps.append(dict(
            mega_in=mega, WsT16=WsT16, WeT16=WeT16, Wd8p=Wd8p,
            wts16=wts16, wte16=wte16, wtd8p=wtd8p,
            arep16=arep16, ident16=ident16,
        ))
        node_maps.append((t_of_node, s_of_node))
    return in_maps, node_maps, T_tiles


# ------------------------------------------------------------- bass program

def build_program(T_tiles):
    nc = bacc.Bacc("TRN2", target_bir_lowering=False, debug=False,
                   num_devices=N_CORES)
    n_ch = T_tiles // CH_TILES

    mega_d = nc.dram_tensor("mega_in", [P, n_ch * CH_BYTES], U8,
                            kind="ExternalInput")
    WsT_d = nc.dram_tensor("WsT16", [P, IN_FEAT], F16, kind="ExternalInput")
    WeT_d = nc.dram_tensor("WeT16", [P, IN_FEAT], F16, kind="ExternalInput")
    Wd8_d = nc.dram_tensor("Wd8p", [P, 2 * IN_FEAT], F8, kind="ExternalInput")
    wts_d = nc.dram_tensor("wts16", [P, HEADS], F16, kind="ExternalInput")
    wte_d = nc.dram_tensor("wte16", [P, HEADS], F16, kind="ExternalInput")
    wtd_d = nc.dram_tensor("wtd8p", [P, 2 * HEADS], F8, kind="ExternalInput")
    arep_d = nc.dram_tensor("arep16", [P, IN_FEAT], F16, kind="ExternalInput")
    ident_d = nc.dram_tensor("ident16", [P, P], F16, kind="ExternalInput")
    out_d = nc.dram_tensor("out_sl", [96, n_ch * 408], F16,
                           kind="ExternalOutput")

    with tile.TileContext(nc) as tc:
        with tc.tile_pool(name="const", bufs=1) as cb:
            def cload(name, shape, dt, dram):
                t = cb.tile(shape, dt, name=name)
                nc.sync.dma_start(out=t[:], in_=dram[:])
                return t

            WsT = cload("WsT_s", [P, IN_FEAT], F16, WsT_d)
            WeT = cload("WeT_s", [P, IN_FEAT], F16, WeT_d)
            Wd8 = cload("Wd8_s", [P, 2 * IN_FEAT], F8, Wd8_d)
            wts = cload("wts_s", [P, HEADS], F16, wts_d)
            wte = cload("wte_s", [P, HEADS], F16, wte_d)
            wtd = cload("wtd_s", [P, 2 * HEADS], F8, wtd_d)
            arep = cload("arep_s", [P, IN_FEAT], F16, arep_d)
            ident = cload("ident_s", [P, P], F16, ident_d)

            bias4 = cb.tile([P, 1], FP, name="bias4")
            nc.vector.memset(bias4[:], -EXP_SHIFT)

            Wd8s = Wd8[:].rearrange("p (two f) -> p two f", two=2)
            wtds = wtd[:].rearrange("p (two h) -> p two h", two=2)

            with (
                tc.tile_pool(name="meg", bufs=9) as megp,
                tc.tile_pool(name="t16", bufs=4) as t16p,
                tc.tile_pool(name="ff", bufs=4) as ffp,
                tc.tile_pool(name="msg", bufs=4) as msgp,
                tc.tile_pool(name="u16", bufs=4) as u16p,
                tc.tile_pool(name="fin", bufs=4) as finp,
                tc.tile_pool(name="ps_t", bufs=2, space="PSUM") as pst,
                tc.tile_pool(name="ps_ul", bufs=3, space="PSUM") as psul,
                tc.tile_pool(name="ps_fd", bufs=1, space="PSUM") as psfd,
            ):
                megs, t16s, ffs, uls = {}, {}, {}, {}
                msgs, u16s, o16s = {}, {}, {}

                for c in range(n_ch + 5):
                    j = c - 2             # B1: folds/exp/msg/scat/evacU
                    k2 = c - 3            # B2: fdst + tail
                    k3 = c - 4            # out DMA

                    # ---- deferred slot-row writeback (data ready last iter)
                    if 0 <= k3 < n_ch:
                        nc.scalar.dma_start(
                            out=out_d[:, k3 * 408:(k3 + 1) * 408],
                            in_=o16s.pop(k3)[0:96, :])

                    # ---- mega prefetch (2 iterations ahead)
                    for cc in ([0, 1, 2] if c == 0 else
                               [c + 2] if c + 2 < n_ch else []):
                        meg = megp.tile([P, CH_BYTES], U8, tag="meg")
                        megs[cc] = meg
                        HB = 4 * TILE_BYTES
                        nc.sync.dma_start(
                            out=meg[:, 0:HB],
                            in_=mega_d[:, cc * CH_BYTES:cc * CH_BYTES + HB])
                        nc.sync.dma_start(
                            out=meg[:, HB:CH_BYTES],
                            in_=mega_d[:, cc * CH_BYTES + HB:
                                       (cc + 1) * CH_BYTES])

                    # ---- B1-front: msg mults (chunk j); exp ran last iter
                    if 0 <= j < n_ch:
                        T16j, ULj = t16s.pop(j), uls[j]
                        msg = msgs[j]
                        mv = msg[:].rearrange("p (t f) -> p t f", t=8)
                        exb = mv[:, :, 128:136].unsqueeze(2).to_broadcast(
                            [P, CH_TILES, HEAD_DIM, HEADS])
                        mfeat = msg[:].rearrange(
                            "p (t f) -> p t f", t=8)[:, :, 0:128].rearrange(
                            "p t (d h) -> p t d h", d=16)
                        t16v = T16j[:].rearrange("p (t d h) -> p t d h",
                                                 t=8, d=16)
                        nc.vector.tensor_tensor(
                            out=mfeat[:, 0:5], in0=t16v[:, 0:5],
                            in1=exb[:, 0:5], op=mybir.AluOpType.mult)
                        nc.gpsimd.tensor_tensor(
                            out=mfeat[:, 5:8], in0=t16v[:, 5:8],
                            in1=exb[:, 5:8], op=mybir.AluOpType.mult)

                    # ---- A-compute: T + lin matmuls (chunk c)
                    if c < n_ch:
                        meg = megs[c]
                        T_ps = pst.tile([P, CH_TILES * TILE_E], FP, tag="T")
                        UL = psul.tile([P, 3 * 136 + 64], FP, tag="UL")
                        uls[c] = UL
                        # dummies absorb the psum-free waits so the real
                        # matmuls only wait on the mega DMA
                        nc.tensor.matmul(out=T_ps[:1, 0:1],
                                         lhsT=ident[:, :1], rhs=ident[:, :1],
                                         start=True, stop=True)
                        nc.tensor.matmul(out=UL[:1, 408:409],
                                         lhsT=ident[:, :1], rhs=ident[:, :1],
                                         start=True, stop=True,
                                         skip_group_check=True)
                        for tl in range(CH_TILES):
                            o = tl * TILE_BYTES
                            xs = meg[:, o:o + 256].bitcast(F16)
                            ef = meg[:, o + 256:o + 512].bitcast(F16)
                            xd2 = meg[:, o + 512:o + 640].bitcast(F8) \
                                .unsqueeze(1).to_broadcast([P, 2, TILE_E])
                            ts = slice(tl * TILE_E, (tl + 1) * TILE_E)
                            nc.tensor.matmul(out=T_ps[:, ts], lhsT=xs,
                                             rhs=WsT[:], start=True,
                                             stop=False)
                            nc.tensor.matmul(out=T_ps[:, ts], lhsT=xd2,
                                             rhs=Wd8s, start=False,
                                             stop=False,
                                             perf_mode=mybir.MatmulPerfMode
                                             .DoubleRow)
                            nc.tensor.matmul(out=T_ps[:, ts], lhsT=ef,
                                             rhs=WeT[:], start=False,
                                             stop=True)
                            ls = slice(408 + tl * HEADS,
                                       408 + (tl + 1) * HEADS)
                            nc.tensor.matmul(out=UL[:, ls], lhsT=xs,
                                             rhs=wts[:], start=(tl == 0),
                                             stop=False,
                                             skip_group_check=True)
                            nc.tensor.matmul(out=UL[:, ls], lhsT=xd2,
                                             rhs=wtds, start=False,
                                             stop=False,
                                             perf_mode=mybir.MatmulPerfMode
                                             .DoubleRow,
                                             skip_group_check=True)
                            nc.tensor.matmul(out=UL[:, ls], lhsT=ef,
                                             rhs=wte[:], start=False,
                                             stop=False,
                                             skip_group_check=True)

                    # ---- B1-mid: scatters + U evac to SBUF (chunk j)
                    if 0 <= j < n_ch:
                        megj = megs[j]
                        for tl in range(CH_TILES):
                            sb = tl * TILE_BYTES + 640
                            S1 = megj[:, sb:sb + 64].bitcast(F16)
                            g, o3 = tl // 3, tl % 3
                            nc.tensor.matmul(
                                out=ULj[32 * o3:32 * o3 + 32,
                                        136 * g:136 * g + 136],
                                lhsT=S1, rhs=msg[:, tl * 136:tl * 136 + 136],
                                start=True, stop=True)
                        U16 = u16p.tile([P, 408], F16, tag="U16")
                        u16s[j] = U16
                        nc.scalar.activation(
                            out=U16[:], in_=ULj[:, 0:408],
                            func=mybir.ActivationFunctionType.Copy)
                        uls.pop(j)

                    # ---- B2: fdst matmuls + tail from SBUF (chunk k2)
                    if 0 <= k2 < n_ch:
                        megk = megs.pop(k2)
                        U16k = u16s.pop(k2)
                        msgs.pop(k2)
                        fd_ps = psfd.tile([P, 3 * IN_FEAT], FP, tag="fd")
                        nc.tensor.matmul(out=fd_ps[:1, 0:1],
                                         lhsT=ident[:, :1], rhs=ident[:, :1],
                                         start=True, stop=True)
                        xdn = megk[:, CH_TILES * TILE_BYTES:
                                   CH_TILES * TILE_BYTES + XDN_BYTES] \
                            .bitcast(F8)
                        for g in range(3):
                            xp = xdn[:, P * g:P * g + P] \
                                .unsqueeze(1).to_broadcast([P, 2, P])
                            nc.tensor.matmul(
                                out=fd_ps[:, 128 * g:128 * g + 128],
                                lhsT=xp, rhs=Wd8s, start=True, stop=True,
                                perf_mode=mybir.MatmulPerfMode.DoubleRow)

                        fin = finp.tile([P, 24 + 384 + 408], F16, tag="fin")
                        rz16 = fin[:, 0:24]
                        uo = fin[:, 24:408].rearrange(
                            "p (g d h) -> p g d h", g=3, d=16)
                        o16 = fin[:, 408:816]
                        u16f = U16k[:].rearrange("p (g f) -> p g f", g=3)
                        nc.vector.tensor_scalar(
                            out=rz16.rearrange("p (g h) -> p g h", g=3),
                            in0=u16f[:, :, 128:136],
                            scalar1=EPS_Z, scalar2=None,
                            op0=mybir.AluOpType.max)
                        with nc.allow_low_precision(
                                reason="1/z fits fp16; z >= 2^-14"):
                            nc.vector.reciprocal(out=rz16, in_=rz16)
                        rzb = rz16.rearrange("p (g h) -> p g h", g=3) \
                            .unsqueeze(2).to_broadcast([P, 3, 16, HEADS])
                        nc.vector.tensor_tensor(
                            out=uo,
                            in0=u16f[:, :, 0:128].rearrange(
                                "p g (d h) -> p g d h", d=16),
                            in1=rzb, op=mybir.AluOpType.mult)
                        o16v = o16.rearrange("p (g f) -> p g f", g=3, f=136)
                        nc.vector.tensor_tensor(
                            out=o16v[:, :, 0:128].rearrange(
                                "p g (d h) -> p g d h", d=16),
                            in0=fin[:, 24:408].rearrange(
                                "p (g d h) -> p g d h", g=3, d=16),
                            in1=fd_ps[:].rearrange("p (g d h) -> p g d h",
                                                   g=3, d=16),
                            op=mybir.AluOpType.subtract),
                        nc.vector.tensor_scalar(
                            out=o16v[:, :, 0:128], in0=o16v[:, :, 0:128],
                            scalar1=0.0, scalar2=None,
                            op0=mybir.AluOpType.max)
                        nc.gpsimd.tensor_scalar(
                            out=o16v[:, :, 128:136],
                            in0=u16f[:, :, 128:136],
                            scalar1=0.0, scalar2=None,
                            op0=mybir.AluOpType.add)
                        o16s[k2] = o16

                    # ---- folds + exp for chunk c-1 (end of iteration)
                    fE = c - 1
                    if 0 <= fE < n_ch:
                        FFe = ffs.pop(fE)
                        ULe = uls[fE]
                        line = ULe[:, 408:472]
                        Fv = FFe[:].rearrange("p (t d h) -> p t d h",
                                              t=8, d=16)
                        for d in range(HEAD_DIM):
                            nc.tensor.matmul(
                                out=line, lhsT=ident[:], rhs=Fv[:, :, d, :],
                                start=False, stop=(d == HEAD_DIM - 1),
                                skip_group_check=True)
                        msge = msgp.tile([P, CH_TILES * 136], F16,
                                         tag="msg")
                        msgs[fE] = msge
                        nc.scalar.activation(
                            out=msge[:].rearrange("p (t f) -> p t f",
                                                  t=8)[:, :, 128:136],
                            in_=line.rearrange("p (t h) -> p t h", t=8),
                            func=mybir.ActivationFunctionType.Exp,
                            bias=bias4[:], scale=1.0)
                    # ---- A-evac: T16, |T|, F (chunk c)
                    if c < n_ch:
                        T16 = t16p.tile([P, CH_TILES * TILE_E], F16,
                                        tag="T16")
                        t16s[c] = T16
                        nc.scalar.activation(
                            out=T16[:], in_=T_ps[:],
                            func=mybir.ActivationFunctionType.Copy)
                        FF = ffp.tile([P, CH_TILES * TILE_E], F16, tag="FF")
                        ffs[c] = FF
                        nc.vector.tensor_scalar(
                            out=FF[:].bitcast(mybir.dt.int16),
                            in0=T16[:].bitcast(mybir.dt.int16),
                            scalar1=0x7FFF, scalar2=None,
                            op0=mybir.AluOpType.bitwise_and)
                        nc.vector.tensor_tensor(
                            out=FF[:].rearrange("p (t f) -> p t f", t=8),
                            in0=FF[:].rearrange("p (t f) -> p t f", t=8),
                            in1=arep[:].unsqueeze(1).to_broadcast(
                                [P, CH_TILES, IN_FEAT]),
                            op=mybir.AluOpType.mult)

    nc.compile()
    return nc


_PROGRAM_CACHE = {}


def kernel(**inputs) -> np.ndarray:
    in_maps, node_maps, T_tiles = _prep_cores(**inputs)
    if T_tiles not in _PROGRAM_CACHE:
        _PROGRAM_CACHE[T_tiles] = build_program(T_tiles)
    nc = _PROGRAM_CACHE[T_tiles]
    res = run_bass_kernel_spmd(nc, in_maps, list(range(N_CORES)))

    n_ch = T_tiles // CH_TILES
    inv = np.empty_like(PJ)
    inv[PJ] = np.arange(IN_FEAT)      # vals col j -> feature PJ[j]
    outs = []
    for k in range(N_CORES):
        sl = np.asarray(res.results[k]["out_sl"])      # [96, n_ch*408] f16
        sl = sl.reshape(96, n_ch, 3, 136)
        t_of_node, s_of_node = node_maps[k]
        c = t_of_node // CH_TILES
        tl = t_of_node % CH_TILES
        part = 32 * (tl % 3) + s_of_node
        g = tl // 3
        rows = sl[part, c, g, :]                        # [6250, 136]
        vals = rows[:, 0:128]
        zv = rows[:, (128 + np.arange(IN_FEAT) % 8)]
        vals = np.where(zv > 0, vals, np.float16(0.0))
        outs.append(vals[:, inv].astype(np.float32))
    return np.concatenate(outs, axis=0)
